# revision 11
# baseline (speedup 1.0000x reference)
"""Trainium2 Bass kernel for nn_Attention_73375221285454.

Multi-head self-attention (B=4, N=2048, D=768, H=12, DH=64) with key-padding
mask, distributed over 8 NeuronCores.

Sharding: core c handles batch b = c//2 and half of that batch's UNMASKED
query rows (qh = c%2). Each core computes K/V for its batch's unmasked keys
and attention + output projection for its query share; the 8 outputs cover
all unmasked rows. Rows with a masked query get the batch's uniform-softmax
row (mean over ALL keys of V, then @ Wo), which the host computes directly
(two 768-dim GEMVs per batch) and scatters during unsharding.

Host marshalling per core: keys sorted so unmasked keys come first (attention
is permutation-invariant over keys; the additive -30000 mask table is sorted
identically, so trailing all-masked key tiles are skipped exactly). Unmasked
queries are gathered/split between the core pair. x and all weights are cast
to bfloat16 (PE runs 1 cycle/row for bf16 vs 2+ for fp32; PSUM accumulation
stays fp32 so only operand rounding is lost; measured end-to-end max-rel
~6e-3 vs the 2e-2 gate).

Device schedule per core (all matmul operands bf16, PSUM fp32): V projection
runs first as one dense matmul burst (also ramps the PE p-state), then K/Q
projection for head-pair 0, then a single merged instruction stream where
each head's attention loop has the NEXT pair's K/Q projection matmuls
interleaved between the S and O matmuls — the PE never idles waiting on the
scalar engine's exp, so it stays at the top p-state clock. Output projection
runs as a tail phase.

  per head h, active key tile jt:
    S^T[j, i] = K_h^T.T @ Q_h^T                (PSUM [128, nq])
    P^T       = exp(0.125*S^T + cmneg[j])      (ACT; bf16 out; cmneg=-30000)
    O^T      += vaug[jt, h].T @ P^T            (PSUM [66, nq]; row 64 = s[i])
  r = 1/s on DVE (vector.reciprocal), broadcast on GpSimd, normalize while
  copying out of PSUM (vector multiply, bf16 out into attnT).
  out  = attnT.T @ Wo  (fp32 out rows, DMA per query tile)

PSUM: psS is one [128, 2*nq] tile used with even/odd-jt base offsets (3
banks), psO pool 2 bufs (4 banks), shared work tile for projections (1 bank).

No max-subtraction is needed: logits are ~N(0,1) (exp can't overflow) and
masked keys get exp(logit - 30000) == 0 exactly.
"""

import sys

sys.path.insert(0, "/opt/trn_rl_repo")

import ml_dtypes
import numpy as np

import concourse.bass as bass  # noqa: F401
import concourse.mybir as mybir
import concourse.tile as tile  # noqa: F401
from concourse import bacc
from concourse.bass_utils import run_bass_kernel_spmd

P = 128
B, N, D = 4, 2048, 768
H, DH = 12, 64
DC = D // P              # 6 contraction chunks
SCALE = DH ** -0.5       # 0.125
MASK_NEG = -30000.0

f32 = mybir.dt.float32
bf16 = mybir.dt.bfloat16
np_bf16 = ml_dtypes.bfloat16

_BUILD_CACHE = {}


def build(njt: int, niq: int) -> "bacc.Bacc":
    """Build the SPMD program. njt = key tiles containing any unmasked key;
    niq = query tiles needed for this core's share of unmasked queries."""
    key = (njt, niq)
    if key in _BUILD_CACHE:
        return _BUILD_CACHE[key]

    nk = njt * P             # active key columns
    nq = niq * P             # query rows computed on this core

    def chunks(total, width=512):
        return [(off, min(width, total - off)) for off in range(0, total, width)]

    nc = bacc.Bacc()
    xkT_d = nc.declare_dram_parameter("xkT", [D, nk], bf16, isOutput=False)
    xqT_d = nc.declare_dram_parameter("xqT", [D, nq], bf16, isOutput=False)
    wq_d = nc.declare_dram_parameter("Wq", [D, D], bf16, isOutput=False)
    wk_d = nc.declare_dram_parameter("Wk", [D, D], bf16, isOutput=False)
    wv_d = nc.declare_dram_parameter("Wv", [D, D], bf16, isOutput=False)
    wo_d = nc.declare_dram_parameter("Wo", [D, D], bf16, isOutput=False)
    # cmnegT[p, t] = 0.0 if key (t*128+p) unmasked else -30000.0
    cmneg_d = nc.declare_dram_parameter("cmnegT", [P, njt], f32, isOutput=False)
    out_d = nc.declare_dram_parameter("out", [nq, D], f32, isOutput=True)

    xkT_r = xkT_d.rearrange("(c p) n -> p c n", p=P)
    xqT_r = xqT_d.rearrange("(c p) n -> p c n", p=P)
    wv_r = wv_d.rearrange("(c p) e -> p c e", p=P)
    wq_r = wq_d.rearrange("(c p) e -> p c e", p=P)
    wk_r = wk_d.rearrange("(c p) e -> p c e", p=P)
    wo_r = wo_d.rearrange("(c p) e -> p c e", p=P)

    with tile.TileContext(nc) as tc:
        with tc.tile_pool(name="persist", bufs=1) as persist:
            cmneg = persist.tile([P, njt], f32)
            nc.sync.dma_start(out=cmneg, in_=cmneg_d.ap())
            ones_b = persist.tile([P, H], bf16)
            nc.vector.memset(ones_b, 1.0)

            qT = persist.tile([P, DC, nq], bf16)
            kT = persist.tile([P, DC, nk], bf16)
            vaug = persist.tile([P, njt, H, DH + 2], bf16)
            attnT = persist.tile([P, DC, nq], bf16)
            wv_sb = persist.tile([P, DC, D], bf16)
            wq_sb = persist.tile([P, DC, D], bf16)
            wk_sb = persist.tile([P, DC, D], bf16)
            wo_sb = persist.tile([P, DC, D], bf16)
            xqT = persist.tile([P, DC, nq], bf16)
            xkT = persist.tile([P, DC, nk], bf16)

            # All input DMAs issue upfront, ordered by first use; one
            # multi-dim DMA per tensor (one DGE setup each) so the V burst
            # can start ~6us in. xkT is split into key-groups so the first
            # V-projection tiles don't wait for the whole tensor.
            nc.sync.dma_start(out=wv_sb, in_=wv_r)
            for cg, ce in chunks(nk):
                nc.sync.dma_start(
                    out=xkT[:, :, cg : cg + ce], in_=xkT_r[:, :, cg : cg + ce]
                )
            nc.sync.dma_start(out=wk_sb, in_=wk_r)
            nc.sync.dma_start(out=xqT, in_=xqT_r)
            nc.sync.dma_start(out=wq_sb, in_=wq_r)
            nc.sync.dma_start(out=wo_sb, in_=wo_r)

            with tc.tile_pool(name="pts", bufs=5) as pts, \
                 tc.tile_pool(name="nrm", bufs=2) as nrm:

                qch = [(off, min(512, nq - off)) for off in range(0, nq, 512)]

                # ---- projection work-chunk emitters ----
                def proj_chunk(pool, w_sb, src, dst, off, sz):
                    """dst[:, hdt-block cols off:off+sz] = w.T @ src, one
                    512-max chunk through a PSUM pool tile."""
                    ps = pool.tile([P, 512], f32, tag=pool.name + "w")
                    for dc in range(DC):
                        nc.tensor.matmul(
                            ps[:, 0:sz],
                            w_sb[dc],
                            src[:, dc, off : off + sz],
                            start=(dc == 0),
                            stop=(dc == DC - 1),
                        )
                    nc.vector.tensor_copy(dst[:, off : off + sz], ps[:, 0:sz])

                def vproj_chunk(pool, jt, lo, sz, hlo, hn):
                    ps = pool.tile([P, 512], f32, tag=pool.name + "w")
                    for dc in range(DC):
                        nc.tensor.matmul(
                            ps[:, 0:sz],
                            xkT[:, dc, jt * P : (jt + 1) * P],
                            wv_sb[:, dc, lo : lo + sz],
                            start=(dc == 0),
                            stop=(dc == DC - 1),
                        )
                    nc.vector.tensor_copy(
                        vaug[:, jt, hlo : hlo + hn, 0:DH],
                        ps[:, 0:sz].rearrange("p (h d) -> p h d", h=hn),
                    )

                def kq_pair_work(pool, hdt):
                    """Work items: K then Q projection chunks for pair hdt."""
                    wkb = [wk_sb[:, dc, hdt * P : (hdt + 1) * P] for dc in range(DC)]
                    wqb = [wq_sb[:, dc, hdt * P : (hdt + 1) * P] for dc in range(DC)]
                    items = []
                    for off, sz in chunks(nk):
                        items.append(
                            lambda o=off, s=sz: proj_chunk(
                                pool, wkb, xkT, kT[:, hdt, :], o, s
                            )
                        )
                    for off, sz in chunks(nq):
                        items.append(
                            lambda o=off, s=sz: proj_chunk(
                                pool, wqb, xqT, qT[:, hdt, :], o, s
                            )
                        )
                    return items

                # -------- phase 1: V projection burst (ramps PE clock) -------
                # and K/Q projection for head-pair 0, in a 4-deep PSUM scope
                with tc.tile_pool(name="pre", bufs=4, space="PSUM") as prepool:
                    for jt in range(njt):
                        vproj_chunk(prepool, jt, 0, 512, 0, 8)
                        vproj_chunk(prepool, jt, 512, 256, 8, 4)
                        nc.vector.tensor_copy(
                            vaug[:, jt, :, DH : DH + 2],
                            ones_b[:, :, None].to_broadcast([P, H, 2]),
                        )
                    for item in kq_pair_work(prepool, 0):
                        item()

                # -------- phase 3: merged attention + next-pair projections --
                with tc.tile_pool(name="psS", bufs=2, space="PSUM") as psS_pool, \
                     tc.tile_pool(name="psO", bufs=2, space="PSUM") as psO_pool:
                    # projection work chunks share the psS pool rotation; the
                    # [128, nq] shape keeps the tag uniform (only [:, 0:512]
                    # is used by a work chunk)
                    class _SPoolView:
                        name = "psS_pool"

                        @staticmethod
                        def tile(shape, dtype, tag):
                            t = psS_pool.tile([P, nq], f32, tag="psS",
                                              name="psSwork")
                            return t

                    for hdt in range(DC):
                        work = (kq_pair_work(_SPoolView, hdt + 1)
                                if hdt < DC - 1 else [])
                        wi = 0
                        total_iters = 2 * (njt + 2)
                        it_ctr = 0
                        for hh in (0, 1):
                            h = 2 * hdt + hh
                            pbase = DH * hh
                            psO = psO_pool.tile([DH + 2, nq], f32, tag="psO",
                                                name=f"psO{h % 2}")
                            pending = []   # exp'd tiles not yet fed to O
                            for jt in range(njt + 2):
                                cur = None
                                if jt < njt:
                                    psS = psS_pool.tile([P, nq], f32, tag="psS",
                                                        name=f"psS{jt % 2}")
                                    for a, sz in qch:
                                        nc.tensor.matmul(
                                            psS[:, a : a + sz],
                                            kT[pbase : pbase + DH, hdt,
                                               jt * P : (jt + 1) * P],
                                            qT[pbase : pbase + DH, hdt,
                                               a : a + sz],
                                            start=True,
                                            stop=True,
                                        )
                                    cur = (jt, psS)
                                # interleave projection work between S and O
                                while wi < len(work) and \
                                        wi * total_iters <= it_ctr * len(work):
                                    work[wi](); wi += 1
                                it_ctr += 1
                                # O lags exp by 2 so the PE never waits on exp
                                if len(pending) == 2 or (jt >= njt and pending):
                                    pjt, pT = pending.pop(0)
                                    for a, sz in qch:
                                        nc.tensor.matmul(
                                            psO[:, a : a + sz],
                                            vaug[:, pjt, h, :],
                                            pT[:, a : a + sz],
                                            start=(pjt == 0),
                                            stop=(pjt == njt - 1),
                                        )
                                if cur is not None:
                                    jt_c, psS_c = cur
                                    pT = pts.tile([P, nq], bf16, tag="pT")
                                    nc.scalar.activation(
                                        pT,
                                        psS_c,
                                        mybir.ActivationFunctionType.Exp,
                                        bias=cmneg[:, jt_c : jt_c + 1],
                                        scale=SCALE,
                                    )
                                    pending.append((jt_c, pT))
                            if hh == 1:
                                while wi < len(work):
                                    work[wi](); wi += 1
                            # normalize: 1/s on DVE (fast approx — 18 bits is
                            # ample for a softmax denominator), broadcast on
                            # GpSimd, multiply while copying out of PSUM
                            s_sb = nrm.tile([1, nq], f32, tag="s_sb")
                            nc.vector.tensor_copy(s_sb, psO[DH : DH + 1, :])
                            r_row = nrm.tile([1, nq], f32, tag="r_row")
                            nc.vector.reciprocal_approx_fast(r_row, s_sb)
                            rb_sb = nrm.tile([DH, nq], f32, tag="rb_sb")
                            nc.gpsimd.partition_broadcast(rb_sb, r_row,
                                                          channels=DH)
                            nc.vector.tensor_mul(
                                attnT[pbase : pbase + DH, hdt, :],
                                psO[0:DH, :],
                                rb_sb,
                            )

            # ---------------- phase 4: output projection ----------------
            with tc.tile_pool(name="fin", bufs=2) as fin, \
                 tc.tile_pool(name="psF", bufs=2, space="PSUM") as psF_pool:
                for it in range(niq):
                    psF = psF_pool.tile([P, D], f32, tag="psF")
                    for lo, hi in ((0, 512), (512, 768)):
                        for c in range(DC):
                            nc.tensor.matmul(
                                psF[:, lo:hi],
                                attnT[:, c, it * P : (it + 1) * P],
                                wo_sb[:, c, lo:hi],
                                start=(c == 0),
                                stop=(c == DC - 1),
                            )
                    out_sb = fin.tile([P, D], f32, tag="outsb")
                    nc.vector.tensor_copy(out_sb, psF)
                    nc.sync.dma_start(
                        out=out_d.ap()[it * P : (it + 1) * P, :], in_=out_sb
                    )

    nc.compile()
    _BUILD_CACHE[key] = nc
    return nc


def _marshal(x, x_mask, Wq, Wk, Wv, Wo):
    """Build per-core input maps. Returns (in_maps, njt, niq, scatter, urows)."""
    x = np.asarray(x, dtype=np.float32)
    x_mask = np.asarray(x_mask).astype(bool)
    Wb = {}
    for name, W in (("Wq", Wq), ("Wk", Wk), ("Wv", Wv), ("Wo", Wo)):
        Wb[name] = np.ascontiguousarray(
            np.asarray(W, dtype=np.float32).astype(np_bf16)
        )

    korders, kcounts, urows = [], [], []
    qidx_all = []
    for b in range(B):
        korders.append(np.argsort(~x_mask[b], kind="stable"))
        kcounts.append(int(x_mask[b].sum()))
        # uniform-softmax row for masked queries: mean over ALL keys
        mv = (x[b].mean(0) @ np.asarray(Wv, dtype=np.float32))
        urows.append(mv @ np.asarray(Wo, dtype=np.float32))
        qidx_all.append(np.nonzero(x_mask[b])[0])

    njt = max(1, -(-max(kcounts) // P))
    nk = njt * P

    # split each batch's unmasked queries between its two cores
    qsplit = []
    for b in range(B):
        qa = qidx_all[b]
        half = (len(qa) + 1) // 2
        qsplit.append((qa[:half], qa[half:]))
    niq = max(1, -(-max(len(qs[i]) for qs in qsplit for i in (0, 1)) // P))
    nq = niq * P

    in_maps = []
    scatter = []   # per core: (b, q_indices)
    for c in range(8):
        b, qh = c // 2, c % 2
        order = korders[b][:nk]
        qa = qsplit[b][qh]
        pad = np.zeros(nq - len(qa), dtype=qa.dtype)  # row 0 dup, discarded
        qfull = np.concatenate([qa, pad])

        xT = x[b].T  # [768, 2048] view
        cm = np.where(x_mask[b][order], 0.0, MASK_NEG).astype(np.float32)

        in_maps.append({
            "xkT": np.ascontiguousarray(xT[:, order].astype(np_bf16)),
            "xqT": np.ascontiguousarray(xT[:, qfull].astype(np_bf16)),
            "Wq": Wb["Wq"], "Wk": Wb["Wk"], "Wv": Wb["Wv"], "Wo": Wb["Wo"],
            "cmnegT": np.ascontiguousarray(cm.reshape(njt, P).T),
        })
        scatter.append((b, qa))
    return in_maps, njt, niq, scatter, urows


def run(x, x_mask, Wq, Wk, Wv, Wo, trace=False, tmpdir=None):
    """Run on 8 cores; returns (full_output, BassKernelResults)."""
    in_maps, njt, niq, scatter, urows = _marshal(x, x_mask, Wq, Wk, Wv, Wo)
    nc = build(njt, niq)
    res = run_bass_kernel_spmd(
        nc, in_maps, core_ids=list(range(8)), trace=trace, tmpdir=tmpdir
    )
    x_mask = np.asarray(x_mask).astype(bool)
    out = np.empty((B, N, D), dtype=np.float32)
    for b in range(B):
        out[b, ~x_mask[b]] = urows[b]
    for c in range(8):
        b, qa = scatter[c]
        out[b, qa] = res.results[c]["out"][: len(qa)]
    return out, res


def kernel(**inputs) -> np.ndarray:
    out, _ = run(
        inputs["x"], inputs["x_mask"],
        inputs["Wq"], inputs["Wk"], inputs["Wv"], inputs["Wo"],
        trace=False,
    )
    return out


# revision 12
# speedup vs baseline: 1.0800x; 1.0800x over previous
"""Trainium2 Bass kernel for nn_Attention_73375221285454.

Multi-head self-attention (B=4, N=2048, D=768, H=12, DH=64) with key-padding
mask, distributed over 8 NeuronCores.

Sharding: core c handles batch b = c//2 and half of that batch's UNMASKED
query rows (qh = c%2). Each core computes K/V for its batch's unmasked keys
and attention + output projection for its query share; the 8 outputs cover
all unmasked rows. Rows with a masked query get the batch's uniform-softmax
row (mean over ALL keys of V, then @ Wo), which the host computes directly
(two 768-dim GEMVs per batch) and scatters during unsharding.

Host marshalling per core: keys sorted so unmasked keys come first (attention
is permutation-invariant over keys; the additive -30000 mask table is sorted
identically, so trailing all-masked key tiles are skipped exactly). Unmasked
queries are gathered/split between the core pair. x and all weights are cast
to bfloat16 (PE runs 1 cycle/row for bf16 vs 2+ for fp32; PSUM accumulation
stays fp32 so only operand rounding is lost; measured end-to-end max-rel
~6e-3 vs the 2e-2 gate).

Device schedule per core (all matmul operands bf16, PSUM fp32): V projection
runs first as one dense matmul burst (also ramps the PE p-state), then K/Q
projection for head-pair 0, then a single merged instruction stream where
each head's attention loop has the NEXT pair's K/Q projection matmuls
interleaved between the S and O matmuls — the PE never idles waiting on the
scalar engine's exp, so it stays at the top p-state clock. Output projection
runs as a tail phase.

  per head h, active key tile jt:
    S^T[j, i] = K_h^T.T @ Q_h^T                (PSUM [128, nq])
    P^T       = exp(0.125*S^T + cmneg[j])      (ACT; bf16 out; cmneg=-30000)
    O^T      += vaug[jt, h].T @ P^T            (PSUM [66, nq]; row 64 = s[i])
  r = 1/s on DVE (vector.reciprocal), broadcast on GpSimd, normalize while
  copying out of PSUM (vector multiply, bf16 out into attnT).
  out  = attnT.T @ Wo  (fp32 out rows, DMA per query tile)

PSUM: psS is one [128, 2*nq] tile used with even/odd-jt base offsets (3
banks), psO pool 2 bufs (4 banks), shared work tile for projections (1 bank).

No max-subtraction is needed: logits are ~N(0,1) (exp can't overflow) and
masked keys get exp(logit - 30000) == 0 exactly.
"""

import sys

sys.path.insert(0, "/opt/trn_rl_repo")

import ml_dtypes
import numpy as np

import concourse.bass as bass  # noqa: F401
import concourse.mybir as mybir
import concourse.tile as tile  # noqa: F401
from concourse import bacc
from concourse.bass_utils import run_bass_kernel_spmd

P = 128
B, N, D = 4, 2048, 768
H, DH = 12, 64
DC = D // P              # 6 contraction chunks
SCALE = DH ** -0.5       # 0.125
MASK_NEG = -30000.0

f32 = mybir.dt.float32
bf16 = mybir.dt.bfloat16
np_bf16 = ml_dtypes.bfloat16

_BUILD_CACHE = {}


def build(njt: int, nq: int) -> "bacc.Bacc":
    """Build the SPMD program. njt = key tiles containing any unmasked key;
    nq = query rows (max active per core, padded to a multiple of 8 only —
    no 128-tile padding, S/O/exp cost scales with the exact query count)."""
    key = (njt, nq)
    if key in _BUILD_CACHE:
        return _BUILD_CACHE[key]

    nk = njt * P             # active key columns

    def chunks(total, width=512):
        return [(off, min(width, total - off)) for off in range(0, total, width)]

    nc = bacc.Bacc()
    xkT_d = nc.declare_dram_parameter("xkT", [D, nk], bf16, isOutput=False)
    xqT_d = nc.declare_dram_parameter("xqT", [D, nq], bf16, isOutput=False)
    wq_d = nc.declare_dram_parameter("Wq", [D, D], bf16, isOutput=False)
    wk_d = nc.declare_dram_parameter("Wk", [D, D], bf16, isOutput=False)
    wv_d = nc.declare_dram_parameter("Wv", [D, D], bf16, isOutput=False)
    wo_d = nc.declare_dram_parameter("Wo", [D, D], bf16, isOutput=False)
    # cmnegT[p, t] = 0.0 if key (t*128+p) unmasked else -30000.0
    cmneg_d = nc.declare_dram_parameter("cmnegT", [P, njt], f32, isOutput=False)
    out_d = nc.declare_dram_parameter("out", [nq, D], f32, isOutput=True)

    xkT_r = xkT_d.rearrange("(c p) n -> p c n", p=P)
    xqT_r = xqT_d.rearrange("(c p) n -> p c n", p=P)
    wv_r = wv_d.rearrange("(c p) e -> p c e", p=P)
    wq_r = wq_d.rearrange("(c p) e -> p c e", p=P)
    wk_r = wk_d.rearrange("(c p) e -> p c e", p=P)
    wo_r = wo_d.rearrange("(c p) e -> p c e", p=P)

    with tile.TileContext(nc) as tc:
        with tc.tile_pool(name="persist", bufs=1) as persist:
            cmneg = persist.tile([P, njt], f32)
            nc.sync.dma_start(out=cmneg, in_=cmneg_d.ap())
            ones_b = persist.tile([P, H], bf16)
            nc.vector.memset(ones_b, 1.0)

            qT = persist.tile([P, DC, nq], bf16)
            kT = persist.tile([P, DC, nk], bf16)
            vaug = persist.tile([P, njt, H, DH + 2], bf16)
            attnT = persist.tile([P, DC, nq], bf16)
            wv_sb = persist.tile([P, DC, D], bf16)
            wq_sb = persist.tile([P, DC, D], bf16)
            wk_sb = persist.tile([P, DC, D], bf16)
            wo_sb = persist.tile([P, DC, D], bf16)
            xqT = persist.tile([P, DC, nq], bf16)
            xkT = persist.tile([P, DC, nk], bf16)

            # All input DMAs issue upfront, ordered by first use; one
            # multi-dim DMA per tensor (one DGE setup each) so the V burst
            # can start ~6us in. xkT is split into key-groups so the first
            # V-projection tiles don't wait for the whole tensor.
            nc.sync.dma_start(out=wv_sb, in_=wv_r)
            for cg, ce in chunks(nk):
                nc.sync.dma_start(
                    out=xkT[:, :, cg : cg + ce], in_=xkT_r[:, :, cg : cg + ce]
                )
            nc.sync.dma_start(out=wk_sb, in_=wk_r)
            nc.sync.dma_start(out=xqT, in_=xqT_r)
            nc.sync.dma_start(out=wq_sb, in_=wq_r)
            nc.sync.dma_start(out=wo_sb, in_=wo_r)

            with tc.tile_pool(name="pts", bufs=5) as pts, \
                 tc.tile_pool(name="nrm", bufs=2) as nrm:

                qch = [(off, min(512, nq - off)) for off in range(0, nq, 512)]

                # ---- projection work-chunk emitters ----
                def proj_chunk(pool, w_sb, src, dst, off, sz):
                    """dst[:, hdt-block cols off:off+sz] = w.T @ src, one
                    512-max chunk through a PSUM pool tile."""
                    ps = pool.tile([P, 512], f32, tag=pool.name + "w")
                    for dc in range(DC):
                        nc.tensor.matmul(
                            ps[:, 0:sz],
                            w_sb[dc],
                            src[:, dc, off : off + sz],
                            start=(dc == 0),
                            stop=(dc == DC - 1),
                        )
                    nc.vector.tensor_copy(dst[:, off : off + sz], ps[:, 0:sz])

                def vproj_chunk(pool, jt, lo, sz, hlo, hn):
                    ps = pool.tile([P, 512], f32, tag=pool.name + "w")
                    for dc in range(DC):
                        nc.tensor.matmul(
                            ps[:, 0:sz],
                            xkT[:, dc, jt * P : (jt + 1) * P],
                            wv_sb[:, dc, lo : lo + sz],
                            start=(dc == 0),
                            stop=(dc == DC - 1),
                        )
                    nc.vector.tensor_copy(
                        vaug[:, jt, hlo : hlo + hn, 0:DH],
                        ps[:, 0:sz].rearrange("p (h d) -> p h d", h=hn),
                    )

                def kq_pair_work(pool, hdt):
                    """Work items: K then Q projection chunks for pair hdt."""
                    wkb = [wk_sb[:, dc, hdt * P : (hdt + 1) * P] for dc in range(DC)]
                    wqb = [wq_sb[:, dc, hdt * P : (hdt + 1) * P] for dc in range(DC)]
                    items = []
                    for off, sz in chunks(nk):
                        items.append(
                            lambda o=off, s=sz: proj_chunk(
                                pool, wkb, xkT, kT[:, hdt, :], o, s
                            )
                        )
                    for off, sz in chunks(nq):
                        items.append(
                            lambda o=off, s=sz: proj_chunk(
                                pool, wqb, xqT, qT[:, hdt, :], o, s
                            )
                        )
                    return items

                # -------- phase 1: V projection burst (ramps PE clock) -------
                # and K/Q projection for head-pair 0, in a 4-deep PSUM scope
                with tc.tile_pool(name="pre", bufs=4, space="PSUM") as prepool:
                    for jt in range(njt):
                        vproj_chunk(prepool, jt, 0, 512, 0, 8)
                        vproj_chunk(prepool, jt, 512, 256, 8, 4)
                        nc.vector.tensor_copy(
                            vaug[:, jt, :, DH : DH + 2],
                            ones_b[:, :, None].to_broadcast([P, H, 2]),
                        )
                    for item in kq_pair_work(prepool, 0):
                        item()

                # -------- phase 3: merged attention + next-pair projections --
                with tc.tile_pool(name="psS", bufs=2, space="PSUM") as psS_pool, \
                     tc.tile_pool(name="psO", bufs=2, space="PSUM") as psO_pool:
                    # projection work chunks share the psS pool rotation; the
                    # [128, nq] shape keeps the tag uniform (only [:, 0:512]
                    # is used by a work chunk)
                    class _SPoolView:
                        name = "psS_pool"

                        @staticmethod
                        def tile(shape, dtype, tag):
                            t = psS_pool.tile([P, nq], f32, tag="psS",
                                              name="psSwork")
                            return t

                    for hdt in range(DC):
                        work = (kq_pair_work(_SPoolView, hdt + 1)
                                if hdt < DC - 1 else [])
                        wi = 0
                        total_iters = 2 * (njt + 2)
                        it_ctr = 0
                        for hh in (0, 1):
                            h = 2 * hdt + hh
                            pbase = DH * hh
                            psO = psO_pool.tile([DH + 2, nq], f32, tag="psO",
                                                name=f"psO{h % 2}")
                            pending = []   # exp'd tiles not yet fed to O
                            for jt in range(njt + 2):
                                cur = None
                                if jt < njt:
                                    psS = psS_pool.tile([P, nq], f32, tag="psS",
                                                        name=f"psS{jt % 2}")
                                    for a, sz in qch:
                                        nc.tensor.matmul(
                                            psS[:, a : a + sz],
                                            kT[pbase : pbase + DH, hdt,
                                               jt * P : (jt + 1) * P],
                                            qT[pbase : pbase + DH, hdt,
                                               a : a + sz],
                                            start=True,
                                            stop=True,
                                        )
                                    cur = (jt, psS)
                                # interleave projection work between S and O
                                while wi < len(work) and \
                                        wi * total_iters <= it_ctr * len(work):
                                    work[wi](); wi += 1
                                it_ctr += 1
                                # O lags exp by 2 so the PE never waits on exp
                                if len(pending) == 2 or (jt >= njt and pending):
                                    pjt, pT = pending.pop(0)
                                    for a, sz in qch:
                                        nc.tensor.matmul(
                                            psO[:, a : a + sz],
                                            vaug[:, pjt, h, :],
                                            pT[:, a : a + sz],
                                            start=(pjt == 0),
                                            stop=(pjt == njt - 1),
                                        )
                                if cur is not None:
                                    jt_c, psS_c = cur
                                    pT = pts.tile([P, nq], bf16, tag="pT")
                                    nc.scalar.activation(
                                        pT,
                                        psS_c,
                                        mybir.ActivationFunctionType.Exp,
                                        bias=cmneg[:, jt_c : jt_c + 1],
                                        scale=SCALE,
                                    )
                                    pending.append((jt_c, pT))
                            if hh == 1:
                                while wi < len(work):
                                    work[wi](); wi += 1
                            # normalize: 1/s on DVE (fast approx — 18 bits is
                            # ample for a softmax denominator), broadcast on
                            # GpSimd, multiply while copying out of PSUM
                            s_sb = nrm.tile([1, nq], f32, tag="s_sb")
                            nc.vector.tensor_copy(s_sb, psO[DH : DH + 1, :])
                            r_row = nrm.tile([1, nq], f32, tag="r_row")
                            nc.vector.reciprocal_approx_fast(r_row, s_sb)
                            rb_sb = nrm.tile([DH, nq], f32, tag="rb_sb")
                            nc.gpsimd.partition_broadcast(rb_sb, r_row,
                                                          channels=DH)
                            nc.vector.tensor_mul(
                                attnT[pbase : pbase + DH, hdt, :],
                                psO[0:DH, :],
                                rb_sb,
                            )

            # ---------------- phase 4: output projection ----------------
            with tc.tile_pool(name="fin", bufs=2) as fin, \
                 tc.tile_pool(name="psF", bufs=2, space="PSUM") as psF_pool:
                nit = -(-nq // P)
                for it in range(nit):
                    rows = min(P, nq - it * P)
                    psF = psF_pool.tile([P, D], f32, tag="psF")
                    for lo, hi in ((0, 512), (512, 768)):
                        for c in range(DC):
                            nc.tensor.matmul(
                                psF[0:rows, lo:hi],
                                attnT[:, c, it * P : it * P + rows],
                                wo_sb[:, c, lo:hi],
                                start=(c == 0),
                                stop=(c == DC - 1),
                            )
                    out_sb = fin.tile([P, D], f32, tag="outsb")
                    nc.vector.tensor_copy(out_sb[0:rows, :], psF[0:rows, :])
                    nc.sync.dma_start(
                        out=out_d.ap()[it * P : it * P + rows, :],
                        in_=out_sb[0:rows, :],
                    )

    nc.compile()
    _BUILD_CACHE[key] = nc
    return nc


def _marshal(x, x_mask, Wq, Wk, Wv, Wo):
    """Build per-core input maps. Returns (in_maps, njt, niq, scatter, urows)."""
    x = np.asarray(x, dtype=np.float32)
    x_mask = np.asarray(x_mask).astype(bool)
    Wb = {}
    for name, W in (("Wq", Wq), ("Wk", Wk), ("Wv", Wv), ("Wo", Wo)):
        Wb[name] = np.ascontiguousarray(
            np.asarray(W, dtype=np.float32).astype(np_bf16)
        )

    korders, kcounts, urows = [], [], []
    qidx_all = []
    for b in range(B):
        korders.append(np.argsort(~x_mask[b], kind="stable"))
        kcounts.append(int(x_mask[b].sum()))
        # uniform-softmax row for masked queries: mean over ALL keys
        mv = (x[b].mean(0) @ np.asarray(Wv, dtype=np.float32))
        urows.append(mv @ np.asarray(Wo, dtype=np.float32))
        qidx_all.append(np.nonzero(x_mask[b])[0])

    njt = max(1, -(-max(kcounts) // P))
    nk = njt * P

    # split each batch's unmasked queries between its two cores
    qsplit = []
    for b in range(B):
        qa = qidx_all[b]
        half = (len(qa) + 1) // 2
        qsplit.append((qa[:half], qa[half:]))
    nq = max(8, -(-max(len(qs[i]) for qs in qsplit for i in (0, 1)) // 8) * 8)

    in_maps = []
    scatter = []   # per core: (b, q_indices)
    for c in range(8):
        b, qh = c // 2, c % 2
        order = korders[b][:nk]
        qa = qsplit[b][qh]
        pad = np.zeros(nq - len(qa), dtype=qa.dtype)  # row 0 dup, discarded
        qfull = np.concatenate([qa, pad])

        xT = x[b].T  # [768, 2048] view
        cm = np.where(x_mask[b][order], 0.0, MASK_NEG).astype(np.float32)

        in_maps.append({
            "xkT": np.ascontiguousarray(xT[:, order].astype(np_bf16)),
            "xqT": np.ascontiguousarray(xT[:, qfull].astype(np_bf16)),
            "Wq": Wb["Wq"], "Wk": Wb["Wk"], "Wv": Wb["Wv"], "Wo": Wb["Wo"],
            "cmnegT": np.ascontiguousarray(cm.reshape(njt, P).T),
        })
        scatter.append((b, qa))
    return in_maps, njt, nq, scatter, urows


def run(x, x_mask, Wq, Wk, Wv, Wo, trace=False, tmpdir=None):
    """Run on 8 cores; returns (full_output, BassKernelResults)."""
    in_maps, njt, nq, scatter, urows = _marshal(x, x_mask, Wq, Wk, Wv, Wo)
    nc = build(njt, nq)
    res = run_bass_kernel_spmd(
        nc, in_maps, core_ids=list(range(8)), trace=trace, tmpdir=tmpdir
    )
    x_mask = np.asarray(x_mask).astype(bool)
    out = np.empty((B, N, D), dtype=np.float32)
    for b in range(B):
        out[b, ~x_mask[b]] = urows[b]
    for c in range(8):
        b, qa = scatter[c]
        out[b, qa] = res.results[c]["out"][: len(qa)]
    return out, res


def kernel(**inputs) -> np.ndarray:
    out, _ = run(
        inputs["x"], inputs["x_mask"],
        inputs["Wq"], inputs["Wk"], inputs["Wv"], inputs["Wo"],
        trace=False,
    )
    return out


# revision 13
# speedup vs baseline: 1.0852x; 1.0048x over previous
"""Trainium2 Bass kernel for nn_Attention_73375221285454.

Multi-head self-attention (B=4, N=2048, D=768, H=12, DH=64) with key-padding
mask, distributed over 8 NeuronCores.

Sharding: core c handles batch b = c//2 and half of that batch's UNMASKED
query rows (qh = c%2). Each core computes K/V for its batch's unmasked keys
and attention + output projection for its query share; the 8 outputs cover
all unmasked rows. Rows with a masked query get the batch's uniform-softmax
row (mean over ALL keys of V, then @ Wo), which the host computes directly
(two 768-dim GEMVs per batch) and scatters during unsharding.

Host marshalling per core: keys sorted so unmasked keys come first (attention
is permutation-invariant over keys; the additive -30000 mask table is sorted
identically, so trailing all-masked key tiles are skipped exactly). Unmasked
queries are gathered/split between the core pair. x and all weights are cast
to bfloat16 (PE runs 1 cycle/row for bf16 vs 2+ for fp32; PSUM accumulation
stays fp32 so only operand rounding is lost; measured end-to-end max-rel
~6e-3 vs the 2e-2 gate).

Device schedule per core (all matmul operands bf16, PSUM fp32): V projection
runs first as one dense matmul burst (also ramps the PE p-state), then K/Q
projection for head-pair 0, then a single merged instruction stream where
each head's attention loop has the NEXT pair's K/Q projection matmuls
interleaved between the S and O matmuls — the PE never idles waiting on the
scalar engine's exp, so it stays at the top p-state clock. Output projection
runs as a tail phase.

  per head h, active key tile jt:
    S^T[j, i] = K_h^T.T @ Q_h^T                (PSUM [128, nq])
    P^T       = exp(0.125*S^T + cmneg[j])      (ACT; bf16 out; cmneg=-30000)
    O^T      += vaug[jt, h].T @ P^T            (PSUM [66, nq]; row 64 = s[i])
  r = 1/s on DVE (vector.reciprocal), broadcast on GpSimd, normalize while
  copying out of PSUM (vector multiply, bf16 out into attnT).
  out  = attnT.T @ Wo  (fp32 out rows, DMA per query tile)

PSUM: psS is one [128, 2*nq] tile used with even/odd-jt base offsets (3
banks), psO pool 2 bufs (4 banks), shared work tile for projections (1 bank).

No max-subtraction is needed: logits are ~N(0,1) (exp can't overflow) and
masked keys get exp(logit - 30000) == 0 exactly.
"""

import sys

sys.path.insert(0, "/opt/trn_rl_repo")

import ml_dtypes
import numpy as np

import concourse.bass as bass  # noqa: F401
import concourse.mybir as mybir
import concourse.tile as tile  # noqa: F401
from concourse import bacc
from concourse.bass_utils import run_bass_kernel_spmd

P = 128
B, N, D = 4, 2048, 768
H, DH = 12, 64
DC = D // P              # 6 contraction chunks
SCALE = DH ** -0.5       # 0.125
MASK_NEG = -30000.0

f32 = mybir.dt.float32
bf16 = mybir.dt.bfloat16
np_bf16 = ml_dtypes.bfloat16

_BUILD_CACHE = {}


def build(njt: int, nq: int) -> "bacc.Bacc":
    """Build the SPMD program. njt = key tiles containing any unmasked key;
    nq = query rows (max active per core, padded to a multiple of 8 only —
    no 128-tile padding, S/O/exp cost scales with the exact query count)."""
    key = (njt, nq)
    if key in _BUILD_CACHE:
        return _BUILD_CACHE[key]

    nk = njt * P             # active key columns

    def chunks(total, width=512):
        return [(off, min(width, total - off)) for off in range(0, total, width)]

    nc = bacc.Bacc()
    xkT_d = nc.declare_dram_parameter("xkT", [D, nk], bf16, isOutput=False)
    xqT_d = nc.declare_dram_parameter("xqT", [D, nq], bf16, isOutput=False)
    wq_d = nc.declare_dram_parameter("Wq", [D, D], bf16, isOutput=False)
    wk_d = nc.declare_dram_parameter("Wk", [D, D], bf16, isOutput=False)
    wv_d = nc.declare_dram_parameter("Wv", [D, D], bf16, isOutput=False)
    wo_d = nc.declare_dram_parameter("Wo", [D, D], bf16, isOutput=False)
    # cmnegT[p, t] = 0.0 if key (t*128+p) unmasked else -30000.0
    cmneg_d = nc.declare_dram_parameter("cmnegT", [P, njt], f32, isOutput=False)
    out_d = nc.declare_dram_parameter("out", [nq, D], bf16, isOutput=True)

    xkT_r = xkT_d.rearrange("(c p) n -> p c n", p=P)
    xqT_r = xqT_d.rearrange("(c p) n -> p c n", p=P)
    wv_r = wv_d.rearrange("(c p) e -> p c e", p=P)
    wq_r = wq_d.rearrange("(c p) e -> p c e", p=P)
    wk_r = wk_d.rearrange("(c p) e -> p c e", p=P)
    wo_r = wo_d.rearrange("(c p) e -> p c e", p=P)

    with tile.TileContext(nc) as tc:
        with tc.tile_pool(name="persist", bufs=1) as persist:
            cmneg = persist.tile([P, njt], f32)
            nc.sync.dma_start(out=cmneg, in_=cmneg_d.ap())
            ones_b = persist.tile([P, H], bf16)
            nc.vector.memset(ones_b, 1.0)

            qT = persist.tile([P, DC, nq], bf16)
            kT = persist.tile([P, DC, nk], bf16)
            vaug = persist.tile([P, njt, H, DH + 2], bf16)
            attnT = persist.tile([P, DC, nq], bf16)
            wv_sb = persist.tile([P, DC, D], bf16)
            wq_sb = persist.tile([P, DC, D], bf16)
            wk_sb = persist.tile([P, DC, D], bf16)
            wo_sb = persist.tile([P, DC, D], bf16)
            xqT = persist.tile([P, DC, nq], bf16)
            xkT = persist.tile([P, DC, nk], bf16)

            # All input DMAs issue upfront, ordered by first use; one
            # multi-dim DMA per tensor (one DGE setup each) so the V burst
            # can start ~6us in. xkT is split into key-groups so the first
            # V-projection tiles don't wait for the whole tensor.
            for dc in range(DC):
                nc.sync.dma_start(out=wv_sb[:, dc, :], in_=wv_r[:, dc, :])
                nc.sync.dma_start(out=xkT[:, dc, 0:512], in_=xkT_r[:, dc, 0:512])
            for cg, ce in chunks(nk):
                if cg == 0:
                    continue
                nc.sync.dma_start(
                    out=xkT[:, :, cg : cg + ce], in_=xkT_r[:, :, cg : cg + ce]
                )
            nc.sync.dma_start(out=wk_sb, in_=wk_r)
            nc.sync.dma_start(out=xqT, in_=xqT_r)
            nc.sync.dma_start(out=wq_sb, in_=wq_r)
            nc.sync.dma_start(out=wo_sb, in_=wo_r)

            with tc.tile_pool(name="pts", bufs=5) as pts, \
                 tc.tile_pool(name="nrm", bufs=2) as nrm:

                qch = [(off, min(512, nq - off)) for off in range(0, nq, 512)]

                # ---- projection work-chunk emitters ----
                def proj_chunk(pool, w_sb, src, dst, off, sz):
                    """dst[:, hdt-block cols off:off+sz] = w.T @ src, one
                    512-max chunk through a PSUM pool tile."""
                    ps = pool.tile([P, 512], f32, tag=pool.name + "w")
                    for dc in range(DC):
                        nc.tensor.matmul(
                            ps[:, 0:sz],
                            w_sb[dc],
                            src[:, dc, off : off + sz],
                            start=(dc == 0),
                            stop=(dc == DC - 1),
                        )
                    nc.vector.tensor_copy(dst[:, off : off + sz], ps[:, 0:sz])

                def vproj_chunk(pool, jt, lo, sz, hlo, hn):
                    ps = pool.tile([P, 512], f32, tag=pool.name + "w")
                    for dc in range(DC):
                        nc.tensor.matmul(
                            ps[:, 0:sz],
                            xkT[:, dc, jt * P : (jt + 1) * P],
                            wv_sb[:, dc, lo : lo + sz],
                            start=(dc == 0),
                            stop=(dc == DC - 1),
                        )
                    nc.vector.tensor_copy(
                        vaug[:, jt, hlo : hlo + hn, 0:DH],
                        ps[:, 0:sz].rearrange("p (h d) -> p h d", h=hn),
                    )

                def kq_pair_work(pool, hdt):
                    """Work items: K then Q projection chunks for pair hdt."""
                    wkb = [wk_sb[:, dc, hdt * P : (hdt + 1) * P] for dc in range(DC)]
                    wqb = [wq_sb[:, dc, hdt * P : (hdt + 1) * P] for dc in range(DC)]
                    items = []
                    for off, sz in chunks(nk):
                        items.append(
                            lambda o=off, s=sz: proj_chunk(
                                pool, wkb, xkT, kT[:, hdt, :], o, s
                            )
                        )
                    for off, sz in chunks(nq):
                        items.append(
                            lambda o=off, s=sz: proj_chunk(
                                pool, wqb, xqT, qT[:, hdt, :], o, s
                            )
                        )
                    return items

                # -------- phase 1: V projection burst (ramps PE clock) -------
                # and K/Q projection for head-pair 0, in a 4-deep PSUM scope
                with tc.tile_pool(name="pre", bufs=4, space="PSUM") as prepool:
                    for jt in range(njt):
                        vproj_chunk(prepool, jt, 0, 512, 0, 8)
                        vproj_chunk(prepool, jt, 512, 256, 8, 4)
                        nc.vector.tensor_copy(
                            vaug[:, jt, :, DH : DH + 2],
                            ones_b[:, :, None].to_broadcast([P, H, 2]),
                        )
                    for item in kq_pair_work(prepool, 0):
                        item()

                # -------- phase 3: merged attention + next-pair projections --
                with tc.tile_pool(name="psS", bufs=2, space="PSUM") as psS_pool, \
                     tc.tile_pool(name="psO", bufs=2, space="PSUM") as psO_pool:
                    # projection work chunks share the psS pool rotation; the
                    # [128, nq] shape keeps the tag uniform (only [:, 0:512]
                    # is used by a work chunk)
                    class _SPoolView:
                        name = "psS_pool"

                        @staticmethod
                        def tile(shape, dtype, tag):
                            t = psS_pool.tile([P, nq], f32, tag="psS",
                                              name="psSwork")
                            return t

                    for hdt in range(DC):
                        work = (kq_pair_work(_SPoolView, hdt + 1)
                                if hdt < DC - 1 else [])
                        wi = 0
                        total_iters = 2 * (njt + 2)
                        it_ctr = 0
                        for hh in (0, 1):
                            h = 2 * hdt + hh
                            pbase = DH * hh
                            psO = psO_pool.tile([DH + 2, nq], f32, tag="psO",
                                                name=f"psO{h % 2}")
                            pending = []   # exp'd tiles not yet fed to O
                            for jt in range(njt + 2):
                                cur = None
                                if jt < njt:
                                    psS = psS_pool.tile([P, nq], f32, tag="psS",
                                                        name=f"psS{jt % 2}")
                                    for a, sz in qch:
                                        nc.tensor.matmul(
                                            psS[:, a : a + sz],
                                            kT[pbase : pbase + DH, hdt,
                                               jt * P : (jt + 1) * P],
                                            qT[pbase : pbase + DH, hdt,
                                               a : a + sz],
                                            start=True,
                                            stop=True,
                                        )
                                    cur = (jt, psS)
                                # interleave projection work between S and O
                                while wi < len(work) and \
                                        wi * total_iters <= it_ctr * len(work) * 2:
                                    work[wi](); wi += 1
                                it_ctr += 1
                                # O lags exp by 2 so the PE never waits on exp
                                if len(pending) == 2 or (jt >= njt and pending):
                                    pjt, pT = pending.pop(0)
                                    for a, sz in qch:
                                        nc.tensor.matmul(
                                            psO[:, a : a + sz],
                                            vaug[:, pjt, h, :],
                                            pT[:, a : a + sz],
                                            start=(pjt == 0),
                                            stop=(pjt == njt - 1),
                                        )
                                if cur is not None:
                                    jt_c, psS_c = cur
                                    pT = pts.tile([P, nq], bf16, tag="pT")
                                    nc.scalar.activation(
                                        pT,
                                        psS_c,
                                        mybir.ActivationFunctionType.Exp,
                                        bias=cmneg[:, jt_c : jt_c + 1],
                                        scale=SCALE,
                                    )
                                    pending.append((jt_c, pT))
                            if hh == 1:
                                while wi < len(work):
                                    work[wi](); wi += 1
                            # normalize: 1/s on DVE (fast approx — 18 bits is
                            # ample for a softmax denominator), broadcast on
                            # GpSimd, multiply while copying out of PSUM
                            s_sb = nrm.tile([1, nq], f32, tag="s_sb")
                            nc.vector.tensor_copy(s_sb, psO[DH : DH + 1, :])
                            r_row = nrm.tile([1, nq], f32, tag="r_row")
                            nc.vector.reciprocal_approx_fast(r_row, s_sb)
                            rb_sb = nrm.tile([DH, nq], f32, tag="rb_sb")
                            nc.gpsimd.partition_broadcast(rb_sb, r_row,
                                                          channels=DH)
                            nc.vector.tensor_mul(
                                attnT[pbase : pbase + DH, hdt, :],
                                psO[0:DH, :],
                                rb_sb,
                            )

            # ---------------- phase 4: output projection ----------------
            with tc.tile_pool(name="fin", bufs=2) as fin, \
                 tc.tile_pool(name="psF", bufs=2, space="PSUM") as psF_pool:
                nit = -(-nq // P)
                for it in range(nit):
                    rows = min(P, nq - it * P)
                    psF = psF_pool.tile([P, D], f32, tag="psF")
                    for lo, hi in ((0, 512), (512, 768)):
                        for c in range(DC):
                            nc.tensor.matmul(
                                psF[0:rows, lo:hi],
                                attnT[:, c, it * P : it * P + rows],
                                wo_sb[:, c, lo:hi],
                                start=(c == 0),
                                stop=(c == DC - 1),
                            )
                    out_sb = fin.tile([P, D], bf16, tag="outsb")
                    nc.vector.tensor_copy(out_sb[0:rows, :], psF[0:rows, :])
                    nc.sync.dma_start(
                        out=out_d.ap()[it * P : it * P + rows, :],
                        in_=out_sb[0:rows, :],
                    )

    nc.compile()
    _BUILD_CACHE[key] = nc
    return nc


def _marshal(x, x_mask, Wq, Wk, Wv, Wo):
    """Build per-core input maps. Returns (in_maps, njt, niq, scatter, urows)."""
    x = np.asarray(x, dtype=np.float32)
    x_mask = np.asarray(x_mask).astype(bool)
    Wb = {}
    for name, W in (("Wq", Wq), ("Wk", Wk), ("Wv", Wv), ("Wo", Wo)):
        Wb[name] = np.ascontiguousarray(
            np.asarray(W, dtype=np.float32).astype(np_bf16)
        )

    korders, kcounts, urows = [], [], []
    qidx_all = []
    for b in range(B):
        korders.append(np.argsort(~x_mask[b], kind="stable"))
        kcounts.append(int(x_mask[b].sum()))
        # uniform-softmax row for masked queries: mean over ALL keys
        mv = (x[b].mean(0) @ np.asarray(Wv, dtype=np.float32))
        urows.append(mv @ np.asarray(Wo, dtype=np.float32))
        qidx_all.append(np.nonzero(x_mask[b])[0])

    njt = max(1, -(-max(kcounts) // P))
    nk = njt * P

    # split each batch's unmasked queries between its two cores
    qsplit = []
    for b in range(B):
        qa = qidx_all[b]
        half = (len(qa) + 1) // 2
        qsplit.append((qa[:half], qa[half:]))
    nq = max(8, -(-max(len(qs[i]) for qs in qsplit for i in (0, 1)) // 8) * 8)

    in_maps = []
    scatter = []   # per core: (b, q_indices)
    for c in range(8):
        b, qh = c // 2, c % 2
        order = korders[b][:nk]
        qa = qsplit[b][qh]
        pad = np.zeros(nq - len(qa), dtype=qa.dtype)  # row 0 dup, discarded
        qfull = np.concatenate([qa, pad])

        xT = x[b].T  # [768, 2048] view
        cm = np.where(x_mask[b][order], 0.0, MASK_NEG).astype(np.float32)

        in_maps.append({
            "xkT": np.ascontiguousarray(xT[:, order].astype(np_bf16)),
            "xqT": np.ascontiguousarray(xT[:, qfull].astype(np_bf16)),
            "Wq": Wb["Wq"], "Wk": Wb["Wk"], "Wv": Wb["Wv"], "Wo": Wb["Wo"],
            "cmnegT": np.ascontiguousarray(cm.reshape(njt, P).T),
        })
        scatter.append((b, qa))
    return in_maps, njt, nq, scatter, urows


def run(x, x_mask, Wq, Wk, Wv, Wo, trace=False, tmpdir=None):
    """Run on 8 cores; returns (full_output, BassKernelResults)."""
    in_maps, njt, nq, scatter, urows = _marshal(x, x_mask, Wq, Wk, Wv, Wo)
    nc = build(njt, nq)
    res = run_bass_kernel_spmd(
        nc, in_maps, core_ids=list(range(8)), trace=trace, tmpdir=tmpdir
    )
    x_mask = np.asarray(x_mask).astype(bool)
    out = np.empty((B, N, D), dtype=np.float32)
    for b in range(B):
        out[b, ~x_mask[b]] = urows[b]
    for c in range(8):
        b, qa = scatter[c]
        out[b, qa] = res.results[c]["out"][: len(qa)].astype(np.float32)
    return out, res


def kernel(**inputs) -> np.ndarray:
    out, _ = run(
        inputs["x"], inputs["x_mask"],
        inputs["Wq"], inputs["Wk"], inputs["Wv"], inputs["Wo"],
        trace=False,
    )
    return out
